# revision 15
# baseline (speedup 1.0000x reference)
"""CNN forward kernel for trn2, v2 (pipelined, engine-balanced).
conv1(3x3,1->32)+pool, conv2(3x3,32->64)+pool, conv3(3x3,64->64),
dense 3136->64, dense 64->10, softmax. Data-parallel over 8 cores.

Structure (per core, N=256 samples):
- conv1: K=91 im2col (host prep), 8 y-out slabs m (y-outs {4m-2..4m+1}),
  M=128=(yA2,yB2,c32), free=(n16,x28) chunks. Pool1: ACT relu-evict of
  even-x + DVE max vs odd-x psum + DVE partition fold (yA) -> H1k[k]
  4-slot-y layout; slot replication via DVE copy.
- conv2: K=128=(s4,c32) on H1k[i], 3 dx passes, M=(ymo2,co64). Pool2:
  ACT evict even-x + DVE max + fold + bias/relu tensor_scalar -> H2k.
- conv3: K=128=(g2,c64) on H2k, 6 passes (dx3,r2), relu+bias evict -> h3t.
- dense1: 28 K=128 slabs -> h4; dense2+softmax as [n,10] psum.
All matmul data f16, psum f32, softmax f32."""
import contextlib
import numpy as np
import concourse.bass as bass
import concourse.tile as tile
from concourse import bacc, mybir

f16 = mybir.dt.float16
f32 = mybir.dt.float32
ALU = mybir.AluOpType
ACTF = mybir.ActivationFunctionType

NPC = 256  # samples per core


# ---------------- host-side prep (numpy) ----------------

def prep_xprep(x):
    n = x.shape[0]
    out = np.zeros((91, n, 28), np.float16)
    xs = x[..., 0].astype(np.float16)
    for dx in range(3):
        for yin in range(30):
            y = yin - 1
            if not (0 <= y < 28):
                continue
            xlo, xhi = max(0, 1 - dx), min(28, 29 - dx)
            out[dx * 30 + yin, :, xlo:xhi] = xs[:, y, xlo + dx - 1:xhi + dx - 1]
    out[90] = 1.0
    return out.reshape(91, n * 28)


def prep_w1l(w1, b1):
    # [91, (kk4, eo2, j4, c32)]; yout = 8*kk - 2 + 2*j + eo
    w = w1[:, :, 0, :].astype(np.float16)  # [dy, dx, c]
    out = np.zeros((91, 4, 2, 4, 32), np.float16)
    for kk in range(4):
        for eo in range(2):
            for j in range(4):
                yout = 8 * kk - 2 + 2 * j + eo
                if not (0 <= yout < 28):
                    continue
                for dx in range(3):
                    for yin in range(30):
                        dy = yin - yout
                        if 0 <= dy <= 2:
                            out[dx * 30 + yin, kk, eo, j, :] = w[dy, dx, :]
                out[90, kk, eo, j, :] = b1.astype(np.float16)
    return out.reshape(91, 8 * 128)


def prep_w2l(w2):
    # [(s4,c32), (dx3, ymo2, co64)]; dy = s - ymo
    wf = w2.astype(np.float16)  # [dy, dx, c, co]
    out = np.zeros((4, 32, 3, 2, 64), np.float16)
    for s in range(4):
        for ymo in range(2):
            dy = s - ymo
            if 0 <= dy <= 2:
                for dx in range(3):
                    out[s, :, dx, ymo, :] = wf[dy, dx, :, :]
    return out.reshape(128, 3 * 128)


def prep_w3l(w3):
    # [(g2,c64), (dx3, r2, ymo2, co64)]; dy = 2r + g - ymo
    wf = w3.astype(np.float16)
    out = np.zeros((2, 64, 3, 2, 2, 64), np.float16)
    for g in range(2):
        for r in range(2):
            for ymo in range(2):
                dy = 2 * r + g - ymo
                if 0 <= dy <= 2:
                    for dx in range(3):
                        out[g, :, dx, r, ymo, :] = wf[dy, dx, :, :]
    return out.reshape(128, 6 * 128)


def prep_wd1l(wd1):
    # [(ymo2, c64), (x7, i34, co64)]; y = 2*i3 + ymo (0 if y == 7)
    wf = wd1.astype(np.float16).reshape(7, 7, 64, 64)  # [y, x, c, co]
    out = np.zeros((2, 64, 7, 4, 64), np.float16)
    for ymo in range(2):
        for i3 in range(4):
            y = 2 * i3 + ymo
            if y <= 6:
                out[ymo, :, :, i3, :] = wf[y, :, :, :].transpose(1, 0, 2)
    return out.reshape(128, 28 * 64)


def prep_wd2l(wd2, bd2):
    out = np.zeros((65, 10), np.float16)
    out[:64] = wd2.astype(np.float16)
    out[64] = bd2.astype(np.float16)
    return out


def prep_weights(inputs):
    return {
        'w1l': prep_w1l(np.asarray(inputs['w1']), np.asarray(inputs['b1'])),
        'w2l': prep_w2l(np.asarray(inputs['w2'])),
        'w3l': prep_w3l(np.asarray(inputs['w3'])),
        'wd1l': prep_wd1l(np.asarray(inputs['wd1'])),
        'wd2l': prep_wd2l(np.asarray(inputs['wd2']), np.asarray(inputs['bd2'])),
        'b2r': np.asarray(inputs['b2']).astype(np.float32)[:, None],
        'b3r': np.tile(np.asarray(inputs['b3']).astype(np.float32), 2)[:, None],
        'bd1r': np.asarray(inputs['bd1']).astype(np.float32)[:, None],
    }


def prep_inputs_for_core(inputs, core, weights=None):
    x = np.asarray(inputs['x'])[core * NPC:(core + 1) * NPC]
    d = dict(weights if weights is not None else prep_weights(inputs))
    d['xprep'] = prep_xprep(x)
    return d


# ---------------- kernel builder ----------------

def build_kernel(taps=()):
    nc = bacc.Bacc("TRN2", target_bir_lowering=False, debug=False)
    N = NPC

    ins = {}
    for name, shape, dt in [
            ("xprep", [91, N * 28], f16), ("w1l", [91, 1024], f16),
            ("w2l", [128, 384], f16), ("w3l", [128, 768], f16),
            ("wd1l", [128, 1792], f16), ("wd2l", [65, 10], f16),
            ("b2r", [64, 1], f32), ("b3r", [128, 1], f32), ("bd1r", [64, 1], f32)]:
        ins[name] = nc.dram_tensor(name, shape, dt, kind="ExternalInput")
    out_d = nc.dram_tensor("out", [N, 10], f32, kind="ExternalOutput")

    tap_shapes = {'h1': [128, 7 * N * 16], 'h2': [128, 5 * N * 9],
                  'h3': [128, 4 * 7 * N], 'h4': [65, N]}
    tap_d = {t: nc.dram_tensor("tap_" + t, tap_shapes[t], f16, kind="ExternalOutput")
             for t in taps}

    with tile.TileContext(nc) as tc:
        ctx = contextlib.ExitStack()
        with ctx:
            persist = ctx.enter_context(tc.tile_pool(name="persist", bufs=1))

            def pt(name, shape, dt=f16):
                return persist.tile(shape, dt, name=name)

            sx = pt("sx", [91, N * 28])
            sw1 = pt("sw1", [91, 1024]); sw2 = pt("sw2", [128, 384])
            sw3 = pt("sw3", [128, 768]); swd1 = pt("swd1", [128, 1792])
            swd2 = pt("swd2", [65, 10])
            sb2 = pt("sb2", [64, 1], f32); sb3 = pt("sb3", [128, 1], f32)
            sbd1 = pt("sbd1", [64, 1], f32)
            H1 = [pt(f"H1_{k}", [128, N * 16]) for k in range(7)]
            H2 = [pt(f"H2_{k}", [128, N * 9]) for k in range(5)]
            h3t = [pt(f"h3_{k}", [128, 7 * N]) for k in range(4)]
            h4 = pt("h4", [65, N])

            # --- input DMA: conv1-critical first on SP queue; rest on ACT queue ---
            nc.sync.dma_start(sw1[:], ins["w1l"].ap())
            for c in range(4):
                nc.sync.dma_start(sx[:, c * 1792:(c + 1) * 1792],
                                  ins["xprep"].ap()[:, c * 1792:(c + 1) * 1792])
            for name, dst in [("w2l", sw2), ("w3l", sw3), ("wd1l", swd1),
                              ("wd2l", swd2), ("b2r", sb2), ("b3r", sb3),
                              ("bd1r", sbd1)]:
                nc.scalar.dma_start(dst[:], ins[name].ap())

            # --- zero-init H1/H2 (contiguous full-tile memsets are cheap;
            # covers all pad slots; odd-k H1 fully DMA-copied later) ---
            for k in (0, 2, 4, 6):
                nc.gpsimd.memset(H1[k][:], 0)
            for k in range(5):
                nc.gpsimd.memset(H2[k][:], 0)
            nc.vector.memset(h4[64:65, :], 1.0)

            pspool = ctx.enter_context(tc.tile_pool(name="ps", bufs=4, space="PSUM"))
            epool = ctx.enter_context(tc.tile_pool(name="ep", bufs=6))
            s2pool = ctx.enter_context(tc.tile_pool(name="s2p", bufs=2))
            tpool = ctx.enter_context(tc.tile_pool(name="tp", bufs=2))
            smpool = ctx.enter_context(tc.tile_pool(name="smp", bufs=2))

            sxv = sx[:].rearrange("p (n x) -> p n x", x=28)

            def psum_ap(ps, off, dims):
                return bass.AP(ps.tensor, ps.offset + off, dims)

            def conv1_slab(kk):
                # writes H1[2*kk] fully: psum partitions (j4, c32) hold
                # yout = 8kk-2+2j+eo; pooled u = 4kk-1+j = slot j @ k=2kk
                wE = sw1[:, (kk * 2 + 0) * 128:(kk * 2 + 0) * 128 + 128]
                wO = sw1[:, (kk * 2 + 1) * 128:(kk * 2 + 1) * 128 + 128]
                h1v5 = H1[2 * kk][:].rearrange(
                    "p (qp t n x) -> p qp t n x", qp=8, t=2, n=16, x=16)
                for qp in range(8):
                    psE = pspool.tile([128, 1024], f32, name="psE", tag="ps")
                    psO = pspool.tile([128, 1024], f32, name="psO", tag="ps")
                    for t in range(2):
                        q = qp * 2 + t
                        rhs = sxv[:, q * 16:(q + 1) * 16, :]
                        nc.tensor.matmul(psE[:, t * 512:t * 512 + 448], wE, rhs,
                                         start=True, stop=True)
                        nc.tensor.matmul(psO[:, t * 512:t * 512 + 448], wO, rhs,
                                         start=True, stop=True)
                    # evict psE (relu), fold vs psO, x-pool
                    Et = epool.tile([128, 896], f16, name="Et", tag="Et")
                    srcE = psum_ap(psE, 0, [[1024, 128], [512, 2], [1, 448]])
                    nc.scalar.activation(Et[:].rearrange(
                        "p (t z) -> p t z", t=2), srcE, ACTF.Relu)
                    Etv = Et[:].rearrange("p (t n v e) -> p t n v e",
                                          t=2, n=16, v=14, e=2)
                    F = epool.tile([128, 448], f16, name="F", tag="F")
                    G = epool.tile([128, 448], f16, name="G", tag="G")
                    Fv = F[:].rearrange("p (t n v) -> p t n v", t=2, n=16, v=14)
                    Gv = G[:].rearrange("p (t n v) -> p t n v", t=2, n=16, v=14)
                    psO_ev = psum_ap(psO, 0, [[1024, 128], [512, 2], [28, 16], [2, 14]])
                    psO_od = psum_ap(psO, 1, [[1024, 128], [512, 2], [28, 16], [2, 14]])
                    nc.vector.tensor_max(Fv, Etv[:, :, :, :, 0], psO_ev)
                    nc.vector.tensor_max(Gv, Etv[:, :, :, :, 1], psO_od)
                    nc.vector.tensor_max(h1v5[:, qp, :, :, 1:15], Fv, Gv)

            def conv2_block(i):
                S2 = s2pool.tile([128, 1792], f16, name="S2", tag="S2")
                h1v = H1[i][:].rearrange("p (n x) -> p n x", x=16)
                S2v4 = S2[:].rearrange("p (np t n v) -> p np t n v",
                                       np=4, t=2, n=32, v=7)
                for np_ in range(4):
                    ps = pspool.tile([128, 1024], f32, name="ps2", tag="ps")
                    for t in range(2):
                        ncb = np_ * 2 + t
                        for dx in range(3):
                            nc.tensor.matmul(
                                ps[:, t * 512:t * 512 + 448],
                                sw2[:, dx * 128:(dx + 1) * 128],
                                h1v[:, ncb * 32:(ncb + 1) * 32, dx:dx + 14],
                                start=(dx == 0), stop=(dx == 2))
                    E2 = epool.tile([128, 448], f16, name="E2", tag="E2")
                    E2v = E2[:].rearrange("p (t n v) -> p t n v", t=2, n=32, v=7)
                    ps_ev = psum_ap(ps, 0, [[1024, 128], [512, 2], [14, 32], [2, 7]])
                    ps_od = psum_ap(ps, 1, [[1024, 128], [512, 2], [14, 32], [2, 7]])
                    nc.scalar.copy(E2v, ps_ev)
                    nc.vector.tensor_max(S2v4[:, np_], E2v, ps_od)
                U = tpool.tile([64, 1792], f16, name="U", tag="U")
                nc.sync.dma_start(U[:], S2[64:128, :])
                T = tpool.tile([64, 1792], f16, name="T", tag="T")
                nc.vector.tensor_max(T[:], S2[0:64, :], U[:])
                Tv = T[:].rearrange("p (n v) -> p n v", v=7)
                if i % 2 == 0:
                    dst = H2[i // 2][64:128, :]
                else:
                    dst = H2[(i + 1) // 2][0:64, :]
                dstv = dst.rearrange("p (n x) -> p n x", x=9)[:, :, 1:8]
                nc.vector.tensor_scalar(dstv, Tv, sb2[:], 0.0, ALU.add, ALU.max)

            def h1_odd_fill(k):
                # H1[k] (k odd) halves are copies of neighbor tiles
                nc.scalar.dma_start(H1[k][0:64, :], H1[k - 1][64:128, :])
                nc.scalar.dma_start(H1[k][64:128, :], H1[k + 1][0:64, :])

            # interleave conv1 / conv2 for engine balance
            conv1_slab(0)
            conv1_slab(1)
            h1_odd_fill(1)
            conv2_block(0)
            conv2_block(1)
            conv1_slab(2)
            h1_odd_fill(3)
            conv2_block(2)
            conv2_block(3)
            conv1_slab(3)
            h1_odd_fill(5)
            conv2_block(4)
            conv2_block(5)
            conv2_block(6)
            if 'h1' in tap_d:
                for k in range(7):
                    nc.sync.dma_start(
                        tap_d['h1'].ap()[:, k * N * 16:(k + 1) * N * 16], H1[k][:])
            if 'h2' in tap_d:
                for k in range(5):
                    nc.sync.dma_start(
                        tap_d['h2'].ap()[:, k * N * 9:(k + 1) * N * 9], H2[k][:])

            # ---------------- conv3 ----------------
            for i3 in range(4):
                for ncb in range(4):
                    ps = pspool.tile([128, 512], f32, name="ps3", tag="ps")
                    for dx in range(3):
                        for r in range(2):
                            h2v = H2[i3 + r][:].rearrange("p (n x) -> p n x", x=9)
                            nc.tensor.matmul(
                                ps[:, 0:448],
                                sw3[:, (dx * 2 + r) * 128:(dx * 2 + r + 1) * 128],
                                h2v[:, ncb * 64:(ncb + 1) * 64, dx:dx + 7],
                                start=(dx == 0 and r == 0),
                                stop=(dx == 2 and r == 1))
                    dst = h3t[i3][:].rearrange(
                        "p (x n) -> p n x", n=N)[:, ncb * 64:(ncb + 1) * 64, :]
                    src = ps[:, 0:448].rearrange("p (n x) -> p n x", x=7)
                    if ncb % 2 == 0:
                        nc.scalar.activation(dst, src, ACTF.Relu, bias=sb3[:])
                    else:
                        nc.vector.tensor_scalar(dst, src, sb3[:], 0.0,
                                                ALU.add, ALU.max)
            if 'h3' in tap_d:
                for k in range(4):
                    nc.sync.dma_start(
                        tap_d['h3'].ap()[:, k * 7 * N:(k + 1) * 7 * N], h3t[k][:])

            # ---------------- dense1 ----------------
            psd = pspool.tile([128, 512], f32, name="psd1", tag="ps")
            first = True
            for i3 in range(4):
                for x in range(7):
                    s = x * 4 + i3
                    rhs = h3t[i3][:].rearrange("p (x n) -> p x n", n=N)[:, x, :]
                    nc.tensor.matmul(psd[0:64, 0:N], swd1[:, s * 64:(s + 1) * 64],
                                     rhs, start=first,
                                     stop=(i3 == 3 and x == 6))
                    first = False
            nc.scalar.activation(h4[0:64, :], psd[0:64, 0:N], ACTF.Relu,
                                 bias=sbd1[:])
            if 'h4' in tap_d:
                nc.sync.dma_start(tap_d['h4'].ap(), h4[:])

            # ---------------- dense2 + softmax ----------------
            for nq in range(2):
                psd2 = pspool.tile([128, 512], f32, name="psd2", tag="ps")
                nc.tensor.matmul(psd2[:, 0:10], h4[:, nq * 128:(nq + 1) * 128],
                                 swd2[:], start=True, stop=True)
                negmx = smpool.tile([128, 1], f32, name="negmx", tag="negmx")
                nc.vector.tensor_reduce(negmx[:], psd2[:, 0:10],
                                        axis=mybir.AxisListType.X, op=ALU.max,
                                        negate=True)
                e = smpool.tile([128, 10], f32, name="e", tag="e")
                se = smpool.tile([128, 1], f32, name="se", tag="se")
                nc.scalar.activation(e[:], psd2[:, 0:10], ACTF.Exp,
                                     bias=negmx[:], scale=1.0, accum_out=se[:])
                rec = smpool.tile([128, 1], f32, name="rec", tag="rec")
                nc.vector.reciprocal(rec[:], se[:])
                osb = smpool.tile([128, 10], f32, name="osb", tag="osb")
                nc.vector.tensor_scalar_mul(osb[:], e[:], rec[:])
                nc.sync.dma_start(out_d.ap()[nq * 128:(nq + 1) * 128, :], osb[:])

    nc.compile()
    return nc


_NC_CACHE = {}


def _get_nc():
    if 'nc' not in _NC_CACHE:
        _NC_CACHE['nc'] = build_kernel()
    return _NC_CACHE['nc']


def kernel(**inputs):
    from concourse.bass_utils import run_bass_kernel_spmd
    nc = _get_nc()
    w = prep_weights(inputs)
    in_maps = [prep_inputs_for_core(inputs, c, weights=w) for c in range(8)]
    res = run_bass_kernel_spmd(nc, in_maps, core_ids=list(range(8)))
    return np.concatenate([res.results[c]['out'] for c in range(8)], axis=0)


# revision 16
# speedup vs baseline: 1.0529x; 1.0529x over previous
"""CNN forward kernel for trn2, v2 (pipelined, engine-balanced).
conv1(3x3,1->32)+pool, conv2(3x3,32->64)+pool, conv3(3x3,64->64),
dense 3136->64, dense 64->10, softmax. Data-parallel over 8 cores.

Structure (per core, N=256 samples):
- conv1: K=91 im2col (host prep), 8 y-out slabs m (y-outs {4m-2..4m+1}),
  M=128=(yA2,yB2,c32), free=(n16,x28) chunks. Pool1: ACT relu-evict of
  even-x + DVE max vs odd-x psum + DVE partition fold (yA) -> H1k[k]
  4-slot-y layout; slot replication via DVE copy.
- conv2: K=128=(s4,c32) on H1k[i], 3 dx passes, M=(ymo2,co64). Pool2:
  ACT evict even-x + DVE max + fold + bias/relu tensor_scalar -> H2k.
- conv3: K=128=(g2,c64) on H2k, 6 passes (dx3,r2), relu+bias evict -> h3t.
- dense1: 28 K=128 slabs -> h4; dense2+softmax as [n,10] psum.
All matmul data f16, psum f32, softmax f32."""
import contextlib
import numpy as np
import concourse.bass as bass
import concourse.tile as tile
from concourse import bacc, mybir

f16 = mybir.dt.float16
f32 = mybir.dt.float32
ALU = mybir.AluOpType
ACTF = mybir.ActivationFunctionType

NPC = 256  # samples per core


# ---------------- host-side prep (numpy) ----------------

def prep_xprep(x):
    n = x.shape[0]
    out = np.zeros((91, n, 28), np.float16)
    xs = x[..., 0].astype(np.float16)
    for dx in range(3):
        for yin in range(30):
            y = yin - 1
            if not (0 <= y < 28):
                continue
            xlo, xhi = max(0, 1 - dx), min(28, 29 - dx)
            out[dx * 30 + yin, :, xlo:xhi] = xs[:, y, xlo + dx - 1:xhi + dx - 1]
    out[90] = 1.0
    return out.reshape(91, n * 28)


def prep_w1l(w1, b1):
    # [91, (kk4, eo2, j4, c32)]; yout = 8*kk - 2 + 2*j + eo
    w = w1[:, :, 0, :].astype(np.float16)  # [dy, dx, c]
    out = np.zeros((91, 4, 2, 4, 32), np.float16)
    for kk in range(4):
        for eo in range(2):
            for j in range(4):
                yout = 8 * kk - 2 + 2 * j + eo
                if not (0 <= yout < 28):
                    continue
                for dx in range(3):
                    for yin in range(30):
                        dy = yin - yout
                        if 0 <= dy <= 2:
                            out[dx * 30 + yin, kk, eo, j, :] = w[dy, dx, :]
                out[90, kk, eo, j, :] = b1.astype(np.float16)
    return out.reshape(91, 8 * 128)


def prep_w2l(w2):
    # [(s4,c32), (dx3, ymo2, co64)]; dy = s - ymo
    wf = w2.astype(np.float16)  # [dy, dx, c, co]
    out = np.zeros((4, 32, 3, 2, 64), np.float16)
    for s in range(4):
        for ymo in range(2):
            dy = s - ymo
            if 0 <= dy <= 2:
                for dx in range(3):
                    out[s, :, dx, ymo, :] = wf[dy, dx, :, :]
    return out.reshape(128, 3 * 128)


def prep_w3l(w3):
    # [(g2,c64), (dx3, r2, ymo2, co64)]; dy = 2r + g - ymo
    wf = w3.astype(np.float16)
    out = np.zeros((2, 64, 3, 2, 2, 64), np.float16)
    for g in range(2):
        for r in range(2):
            for ymo in range(2):
                dy = 2 * r + g - ymo
                if 0 <= dy <= 2:
                    for dx in range(3):
                        out[g, :, dx, r, ymo, :] = wf[dy, dx, :, :]
    return out.reshape(128, 6 * 128)


def prep_wd1l(wd1):
    # [(ymo2, c64), (x7, i34, co64)]; y = 2*i3 + ymo (0 if y == 7)
    wf = wd1.astype(np.float16).reshape(7, 7, 64, 64)  # [y, x, c, co]
    out = np.zeros((2, 64, 7, 4, 64), np.float16)
    for ymo in range(2):
        for i3 in range(4):
            y = 2 * i3 + ymo
            if y <= 6:
                out[ymo, :, :, i3, :] = wf[y, :, :, :].transpose(1, 0, 2)
    return out.reshape(128, 28 * 64)


def prep_wd2l(wd2, bd2):
    out = np.zeros((65, 10), np.float16)
    out[:64] = wd2.astype(np.float16)
    out[64] = bd2.astype(np.float16)
    return out


def prep_weights(inputs):
    return {
        'w1l': prep_w1l(np.asarray(inputs['w1']), np.asarray(inputs['b1'])),
        'w2l': prep_w2l(np.asarray(inputs['w2'])),
        'w3l': prep_w3l(np.asarray(inputs['w3'])),
        'wd1l': prep_wd1l(np.asarray(inputs['wd1'])),
        'wd2l': prep_wd2l(np.asarray(inputs['wd2']), np.asarray(inputs['bd2'])),
        'b2r': np.asarray(inputs['b2']).astype(np.float32)[:, None],
        'b3r': np.tile(np.asarray(inputs['b3']).astype(np.float32), 2)[:, None],
        'bd1r': np.asarray(inputs['bd1']).astype(np.float32)[:, None],
    }


def prep_inputs_for_core(inputs, core, weights=None):
    x = np.asarray(inputs['x'])[core * NPC:(core + 1) * NPC]
    d = dict(weights if weights is not None else prep_weights(inputs))
    d['xprep'] = prep_xprep(x)
    return d


# ---------------- kernel builder ----------------

def build_kernel(taps=()):
    nc = bacc.Bacc("TRN2", target_bir_lowering=False, debug=False)
    N = NPC

    ins = {}
    for name, shape, dt in [
            ("xprep", [91, N * 28], f16), ("w1l", [91, 1024], f16),
            ("w2l", [128, 384], f16), ("w3l", [128, 768], f16),
            ("wd1l", [128, 1792], f16), ("wd2l", [65, 10], f16),
            ("b2r", [64, 1], f32), ("b3r", [128, 1], f32), ("bd1r", [64, 1], f32)]:
        ins[name] = nc.dram_tensor(name, shape, dt, kind="ExternalInput")
    out_d = nc.dram_tensor("out", [N, 10], f32, kind="ExternalOutput")

    tap_shapes = {'h1': [128, 7 * N * 16], 'h2': [128, 5 * N * 9],
                  'h3': [128, 4 * 7 * N], 'h4': [65, N]}
    tap_d = {t: nc.dram_tensor("tap_" + t, tap_shapes[t], f16, kind="ExternalOutput")
             for t in taps}

    with tile.TileContext(nc) as tc:
        ctx = contextlib.ExitStack()
        with ctx:
            persist = ctx.enter_context(tc.tile_pool(name="persist", bufs=1))

            def pt(name, shape, dt=f16):
                return persist.tile(shape, dt, name=name)

            sx = pt("sx", [91, N * 28])
            sw1 = pt("sw1", [91, 1024]); sw2 = pt("sw2", [128, 384])
            sw3 = pt("sw3", [128, 768]); swd1 = pt("swd1", [128, 1792])
            swd2 = pt("swd2", [65, 10])
            sb2 = pt("sb2", [64, 1], f32); sb3 = pt("sb3", [128, 1], f32)
            sbd1 = pt("sbd1", [64, 1], f32)
            H1 = [pt(f"H1_{k}", [128, N * 16]) for k in range(7)]
            H2 = [pt(f"H2_{k}", [128, N * 9]) for k in range(5)]
            h3t = [pt(f"h3_{k}", [128, 7 * N]) for k in range(4)]
            h4 = pt("h4", [65, N])

            # --- input DMA: conv1-critical first on SP queue; rest on ACT queue ---
            nc.sync.dma_start(sw1[:], ins["w1l"].ap())
            for c in range(4):
                nc.sync.dma_start(sx[:, c * 1792:(c + 1) * 1792],
                                  ins["xprep"].ap()[:, c * 1792:(c + 1) * 1792])
            for name, dst in [("w2l", sw2), ("w3l", sw3), ("wd1l", swd1),
                              ("wd2l", swd2), ("b2r", sb2), ("b3r", sb3),
                              ("bd1r", sbd1)]:
                nc.scalar.dma_start(dst[:], ins[name].ap())

            # --- zero-init H1/H2 (contiguous full-tile memsets are cheap;
            # covers all pad slots; odd-k H1 fully DMA-copied later) ---
            for k in (0, 2, 4, 6):
                nc.gpsimd.memset(H1[k][:], 0)
            for k in range(5):
                nc.gpsimd.memset(H2[k][:], 0)
            nc.vector.memset(h4[64:65, :], 1.0)

            pspool = ctx.enter_context(tc.tile_pool(name="ps", bufs=4, space="PSUM"))
            epool = ctx.enter_context(tc.tile_pool(name="ep", bufs=6))
            s2pool = ctx.enter_context(tc.tile_pool(name="s2p", bufs=2))
            tpool = ctx.enter_context(tc.tile_pool(name="tp", bufs=2))
            smpool = ctx.enter_context(tc.tile_pool(name="smp", bufs=2))

            sxv = sx[:].rearrange("p (n x) -> p n x", x=28)

            def psum_ap(ps, off, dims):
                return bass.AP(ps.tensor, ps.offset + off, dims)

            def conv1_slab(kk):
                # writes H1[2*kk] fully: psum partitions (j4, c32) hold
                # yout = 8kk-2+2j+eo; pooled u = 4kk-1+j = slot j @ k=2kk
                wE = sw1[:, (kk * 2 + 0) * 128:(kk * 2 + 0) * 128 + 128]
                wO = sw1[:, (kk * 2 + 1) * 128:(kk * 2 + 1) * 128 + 128]
                h1v5 = H1[2 * kk][:].rearrange(
                    "p (qp t n x) -> p qp t n x", qp=8, t=2, n=16, x=16)
                for qp in range(8):
                    psE = pspool.tile([128, 1024], f32, name="psE", tag="ps")
                    psO = pspool.tile([128, 1024], f32, name="psO", tag="ps")
                    for t in range(2):
                        q = qp * 2 + t
                        rhs = sxv[:, q * 16:(q + 1) * 16, :]
                        nc.tensor.matmul(psE[:, t * 512:t * 512 + 448], wE, rhs,
                                         start=True, stop=True)
                        nc.tensor.matmul(psO[:, t * 512:t * 512 + 448], wO, rhs,
                                         start=True, stop=True)
                    # evict psE (relu), fold vs psO, x-pool
                    Et = epool.tile([128, 896], f16, name="Et", tag="Et")
                    srcE = psum_ap(psE, 0, [[1024, 128], [512, 2], [1, 448]])
                    nc.scalar.activation(Et[:].rearrange(
                        "p (t z) -> p t z", t=2), srcE, ACTF.Relu)
                    Etv = Et[:].rearrange("p (t n v e) -> p t n v e",
                                          t=2, n=16, v=14, e=2)
                    F = epool.tile([128, 448], f16, name="F", tag="F")
                    G = epool.tile([128, 448], f16, name="G", tag="G")
                    Fv = F[:].rearrange("p (t n v) -> p t n v", t=2, n=16, v=14)
                    Gv = G[:].rearrange("p (t n v) -> p t n v", t=2, n=16, v=14)
                    psO_ev = psum_ap(psO, 0, [[1024, 128], [512, 2], [28, 16], [2, 14]])
                    psO_od = psum_ap(psO, 1, [[1024, 128], [512, 2], [28, 16], [2, 14]])
                    nc.vector.tensor_max(Fv, Etv[:, :, :, :, 0], psO_ev)
                    nc.vector.tensor_max(Gv, Etv[:, :, :, :, 1], psO_od)
                    nc.vector.tensor_max(h1v5[:, qp, :, :, 1:15], Fv, Gv)

            def conv2_block(i):
                S2 = s2pool.tile([128, 1792], f16, name="S2", tag="S2")
                h1v = H1[i][:].rearrange("p (n x) -> p n x", x=16)
                S2v4 = S2[:].rearrange("p (np t n v) -> p np t n v",
                                       np=4, t=2, n=32, v=7)
                for np_ in range(4):
                    ps = pspool.tile([128, 1024], f32, name="ps2", tag="ps")
                    for t in range(2):
                        ncb = np_ * 2 + t
                        for dx in range(3):
                            nc.tensor.matmul(
                                ps[:, t * 512:t * 512 + 448],
                                sw2[:, dx * 128:(dx + 1) * 128],
                                h1v[:, ncb * 32:(ncb + 1) * 32, dx:dx + 14],
                                start=(dx == 0), stop=(dx == 2))
                    E2 = epool.tile([128, 448], f16, name="E2", tag="E2")
                    E2v = E2[:].rearrange("p (t n v) -> p t n v", t=2, n=32, v=7)
                    ps_ev = psum_ap(ps, 0, [[1024, 128], [512, 2], [14, 32], [2, 7]])
                    ps_od = psum_ap(ps, 1, [[1024, 128], [512, 2], [14, 32], [2, 7]])
                    nc.scalar.copy(E2v, ps_ev)
                    nc.vector.tensor_max(S2v4[:, np_], E2v, ps_od)
                U = tpool.tile([64, 1792], f16, name="U", tag="U")
                nc.sync.dma_start(U[:], S2[64:128, :])
                T = tpool.tile([64, 1792], f16, name="T", tag="T")
                nc.vector.tensor_max(T[:], S2[0:64, :], U[:])
                Tv = T[:].rearrange("p (n v) -> p n v", v=7)
                if i % 2 == 0:
                    dst = H2[i // 2][64:128, :]
                else:
                    dst = H2[(i + 1) // 2][0:64, :]
                dstv = dst.rearrange("p (n x) -> p n x", x=9)[:, :, 1:8]
                nc.vector.tensor_scalar(dstv, Tv, sb2[:], 0.0, ALU.add, ALU.max)

            def h1_odd_fill(k):
                # H1[k] (k odd) halves are copies of neighbor tiles
                nc.sync.dma_start(H1[k][0:64, :], H1[k - 1][64:128, :])
                nc.sync.dma_start(H1[k][64:128, :], H1[k + 1][0:64, :])

            # interleave conv1 / conv2 for engine balance
            conv1_slab(0)
            conv1_slab(1)
            h1_odd_fill(1)
            conv2_block(0)
            conv2_block(1)
            conv1_slab(2)
            h1_odd_fill(3)
            conv2_block(2)
            conv2_block(3)
            conv1_slab(3)
            h1_odd_fill(5)
            conv2_block(4)
            conv2_block(5)
            conv2_block(6)
            if 'h1' in tap_d:
                for k in range(7):
                    nc.sync.dma_start(
                        tap_d['h1'].ap()[:, k * N * 16:(k + 1) * N * 16], H1[k][:])
            if 'h2' in tap_d:
                for k in range(5):
                    nc.sync.dma_start(
                        tap_d['h2'].ap()[:, k * N * 9:(k + 1) * N * 9], H2[k][:])

            # ---------------- conv3 ----------------
            for i3 in range(4):
                for ncb in range(4):
                    ps = pspool.tile([128, 512], f32, name="ps3", tag="ps")
                    for dx in range(3):
                        for r in range(2):
                            h2v = H2[i3 + r][:].rearrange("p (n x) -> p n x", x=9)
                            nc.tensor.matmul(
                                ps[:, 0:448],
                                sw3[:, (dx * 2 + r) * 128:(dx * 2 + r + 1) * 128],
                                h2v[:, ncb * 64:(ncb + 1) * 64, dx:dx + 7],
                                start=(dx == 0 and r == 0),
                                stop=(dx == 2 and r == 1))
                    dst = h3t[i3][:].rearrange(
                        "p (x n) -> p n x", n=N)[:, ncb * 64:(ncb + 1) * 64, :]
                    src = ps[:, 0:448].rearrange("p (n x) -> p n x", x=7)
                    if ncb % 2 == 0:
                        nc.scalar.activation(dst, src, ACTF.Relu, bias=sb3[:])
                    else:
                        nc.vector.tensor_scalar(dst, src, sb3[:], 0.0,
                                                ALU.add, ALU.max)
            if 'h3' in tap_d:
                for k in range(4):
                    nc.sync.dma_start(
                        tap_d['h3'].ap()[:, k * 7 * N:(k + 1) * 7 * N], h3t[k][:])

            # ---------------- dense1 ----------------
            psd = pspool.tile([128, 512], f32, name="psd1", tag="ps")
            first = True
            for i3 in range(4):
                for x in range(7):
                    s = x * 4 + i3
                    rhs = h3t[i3][:].rearrange("p (x n) -> p x n", n=N)[:, x, :]
                    nc.tensor.matmul(psd[0:64, 0:N], swd1[:, s * 64:(s + 1) * 64],
                                     rhs, start=first,
                                     stop=(i3 == 3 and x == 6))
                    first = False
            nc.scalar.activation(h4[0:64, :], psd[0:64, 0:N], ACTF.Relu,
                                 bias=sbd1[:])
            if 'h4' in tap_d:
                nc.sync.dma_start(tap_d['h4'].ap(), h4[:])

            # ---------------- dense2 + softmax ----------------
            for nq in range(2):
                psd2 = pspool.tile([128, 512], f32, name="psd2", tag="ps")
                nc.tensor.matmul(psd2[:, 0:10], h4[:, nq * 128:(nq + 1) * 128],
                                 swd2[:], start=True, stop=True)
                negmx = smpool.tile([128, 1], f32, name="negmx", tag="negmx")
                nc.vector.tensor_reduce(negmx[:], psd2[:, 0:10],
                                        axis=mybir.AxisListType.X, op=ALU.max,
                                        negate=True)
                e = smpool.tile([128, 10], f32, name="e", tag="e")
                se = smpool.tile([128, 1], f32, name="se", tag="se")
                nc.scalar.activation(e[:], psd2[:, 0:10], ACTF.Exp,
                                     bias=negmx[:], scale=1.0, accum_out=se[:])
                rec = smpool.tile([128, 1], f32, name="rec", tag="rec")
                nc.vector.reciprocal(rec[:], se[:])
                osb = smpool.tile([128, 10], f32, name="osb", tag="osb")
                nc.vector.tensor_scalar_mul(osb[:], e[:], rec[:])
                nc.sync.dma_start(out_d.ap()[nq * 128:(nq + 1) * 128, :], osb[:])

    nc.compile()
    return nc


_NC_CACHE = {}


def _get_nc():
    if 'nc' not in _NC_CACHE:
        _NC_CACHE['nc'] = build_kernel()
    return _NC_CACHE['nc']


def kernel(**inputs):
    from concourse.bass_utils import run_bass_kernel_spmd
    nc = _get_nc()
    w = prep_weights(inputs)
    in_maps = [prep_inputs_for_core(inputs, c, weights=w) for c in range(8)]
    res = run_bass_kernel_spmd(nc, in_maps, core_ids=list(range(8)))
    return np.concatenate([res.results[c]['out'] for c in range(8)], axis=0)
